# revision 36
# baseline (speedup 1.0000x reference)
"""Trainium2 Bass kernel: post-norm transformer block (8-head causal attention
d_model=64 + 64->2048->64 FFN), B=512 T=256, fp32 I/O.

Sharding: pure data-parallel over 8 NeuronCores - 64 sequences per core,
weights replicated. No collectives.

Per-core dataflow (feat-major = feature dim on SBUF partitions, tokens free):
  x [512tok-pair, 64] --PE transpose--> x_fm [64, 512] bf16
  QKV: bf16 matmuls; q/k spread 4-heads-per-128-rows (32-row groups for the
       tile_position packing constraint), v token-major with ONES columns
       interleaved per head (9 cols/head) so each o-matmul also produces the
       softmax row-sums on PSUM row 32g+8 (no separate sums matmuls).
  scoresT[s,t] per head: bf16 matmuls K=8 row-packed via tile_position; exp
       on ScalarE; causal mask via gpsimd affine_select (Pool is the ONLY
       engine that cannot touch PSUM, so it owns the SBUF-side masks);
       softmax denominators for both proj rounds of a sequence live in one
       [128, 2, 256] PSUM tile: one DVE stream_shuffle (lane 8 of each
       32-bank -> all lanes), one reciprocal, one multiply per sequence.
  proj: bf16 matmuls + residual add with x_fm
  LN1/LN2 token-major (PE transpose, bn_stats/bn_aggr on DVE); rstd via
       Ln/Exp on ScalarE; applies on ScalarE as Identity(scale=rstd,
       bias=-mu*rstd) with the scale/bias products prepped on Pool.
  FFN: 16 bf16 W1 chunks row-packed 2x via tile_position rows 0/64 (hhat
       duplicated into both partition halves by an SBUF-SBUF DMA), ReLU
       evictions split ScalarE/DVE, FFN2 bf16 + diag(g1) residual matmul.
"""
import numpy as np
import ml_dtypes

import concourse.bass as bass
import concourse.bacc as bacc
import concourse.tile as tile
from concourse import mybir
from concourse.bass_utils import run_bass_kernel_spmd

dt = mybir.dt
F32 = dt.float32
BF16 = dt.bfloat16
AF = mybir.ActivationFunctionType
OP = mybir.AluOpType
AX = mybir.AxisListType

N_CORES = 8
B, T, D = 512, 256, 64
H, E = 8, 8
HID = 2048
NCHUNK = HID // 128  # 16
S_PER_CORE = B // N_CORES  # 64 sequences/core
NPAIR = S_PER_CORE // 2    # 32 pair iterations
EPS = 1e-5

LAST_RESULTS = None  # test.py reads exec_time_ns from here
REPEAT = 1  # test-only: run the whole body N times in one NEFF for timing
_NC_CACHE = {}

# FFN1 relu eviction engine per chunk pair-slot: 0=ScalarE 1=DVE (6/10)
_RELU_ENG = [1, 0, 1, 1, 0, 1, 1, 0, 1, 0, 1, 1, 0, 1, 1, 0]


def _build_bass():
    # All activation funcs used here (Exp, Ln, Relu, Copy, Identity) live in
    # the one table set "natural_log_exp_and_others". The default assigner
    # binds funcs to different sets and thrashes ~2.7us ACT_TABLE_LOADs;
    # restricting the table list pins a single always-resident set.
    import concourse.bacc as _bacc_mod
    _orig_gat = _bacc_mod.get_activation_tables

    def _one_set(arch):
        tabs = _orig_gat(arch)
        return {name: (fns if name == "natural_log_exp_and_others" else set())
                for name, fns in tabs.items()}

    _bacc_mod.get_activation_tables = _one_set
    try:
        return _build_bass_inner()
    finally:
        _bacc_mod.get_activation_tables = _orig_gat


def _build_bass_inner():
    nc = bacc.Bacc("TRN2", target_bir_lowering=False, debug=False)

    x_d = nc.dram_tensor("x", [S_PER_CORE * T, D], F32, kind="ExternalInput")
    wq_d = nc.dram_tensor("wq_s", [2, D, 128], BF16, kind="ExternalInput")
    wk_d = nc.dram_tensor("wk_s", [2, D, 128], BF16, kind="ExternalInput")
    wv_d = nc.dram_tensor("wv", [D, D], BF16, kind="ExternalInput")
    wp_d = nc.dram_tensor("wp_s", [2, 128, D], BF16, kind="ExternalInput")
    w1_d = nc.dram_tensor("w1f", [128, NCHUNK // 2, 128], BF16,
                          kind="ExternalInput")
    w2_d = nc.dram_tensor("w2r", [NCHUNK, 128, D], BF16, kind="ExternalInput")
    g1d_d = nc.dram_tensor("g1d", [D, D], BF16, kind="ExternalInput")
    id_d = nc.dram_tensor("ident", [128, 128], F32, kind="ExternalInput")
    idb_d = nc.dram_tensor("identb", [128, 128], BF16, kind="ExternalInput")
    out_d = nc.dram_tensor("out", [S_PER_CORE * T, D], F32, kind="ExternalOutput")

    with tile.TileContext(nc) as tc:
        import contextlib
        with contextlib.ExitStack() as ctx:
            _build_body(ctx, tc, nc, x_d, wq_d, wk_d, wv_d, wp_d, w1_d, w2_d,
                        g1d_d, id_d, idb_d, out_d)
    nc.compile()
    return nc


def _build_body(ctx, tc, nc, x_d, wq_d, wk_d, wv_d, wp_d, w1_d, w2_d,
                g1d_d, id_d, idb_d, out_d):
    const = ctx.enter_context(tc.tile_pool(name="const", bufs=1))
    # PSUM: 8 banks. ps = 1-bank hot ring (3 slots); tm = 1-bank (1 slot,
    # long-lived LN-stat tiles ht/zt); psc = 2-bank score tiles (2 slots).
    ps = ctx.enter_context(tc.tile_pool(name="ps", bufs=3, space="PSUM"))
    psc = ctx.enter_context(tc.tile_pool(name="psc", bufs=2, space="PSUM"))
    sbA = ctx.enter_context(tc.tile_pool(name="sbA", bufs=4))
    sbB = ctx.enter_context(tc.tile_pool(name="sbB", bufs=8))
    sbH = ctx.enter_context(tc.tile_pool(name="sbH", bufs=2))

    # ---- constants / weights (persistent, distinct tags in bufs=1 pool) ----
    ident = const.tile([128, 128], F32, tag="ident")
    nc.sync.dma_start(out=ident[:], in_=id_d.ap())
    identb = const.tile([128, 128], BF16, tag="identb")
    nc.sync.dma_start(out=identb[:], in_=idb_d.ap())
    eps_t = const.tile([128, 1], F32, tag="eps_t")
    nc.vector.memset(eps_t[:], EPS)
    # v with ones interleave: col 9h+j = v_h,j (j<8), col 9h+8 = 1.0 so the
    # o-matmul's M=32 window also emits softmax row-sums at out row 32g+8.
    v_aug_bufs = [const.tile([128, 4, 96], BF16, tag=f"v_aug{i}",
                             name=f"v_aug{i}") for i in range(2)]
    for t in v_aug_bufs:
        nc.vector.memset(t[:, :, 72:96], 0.0)
        for h in range(H):
            nc.vector.memset(t[:, :, 9 * h + 8:9 * h + 9], 1.0)

    wq_sb = const.tile([D, 2, 128], BF16, tag="wq_sb")
    nc.sync.dma_start(out=wq_sb[:], in_=wq_d.ap().rearrange("r d m -> d r m"))
    wk_sb = const.tile([D, 2, 128], BF16, tag="wk_sb")
    nc.sync.dma_start(out=wk_sb[:], in_=wk_d.ap().rearrange("r d m -> d r m"))
    wv_sb = const.tile([D, D], BF16, tag="wv_sb")
    nc.sync.dma_start(out=wv_sb[:], in_=wv_d.ap())
    # W1 pre-split into partition halves host-side: rows 0:64 hold chunks
    # 0..7, rows 64:128 hold chunks 8..15 (g1 folded), for 2-concurrent
    # row-tiled FFN1 matmuls (tile_position rows 0/64).
    w1_sb = const.tile([128, NCHUNK // 2, 128], BF16, tag="w1_sb")
    nc.sync.dma_start(out=w1_sb[:], in_=w1_d.ap())
    g1d_sb = const.tile([D, D], BF16, tag="g1d_sb")
    nc.sync.dma_start(out=g1d_sb[:], in_=g1d_d.ap())
    wp_sb = const.tile([128, 2, D], BF16, tag="wp_sb")
    nc.sync.dma_start(out=wp_sb[:], in_=wp_d.ap().rearrange("r p m -> p r m"))
    w2_sb = const.tile([128, NCHUNK, D], BF16, tag="w2_sb")
    nc.sync.dma_start(out=w2_sb[:], in_=w2_d.ap().rearrange("c p m -> p c m"))

    x_ap = x_d.ap()
    out_ap = out_d.ap()

    # per-chunk 2D DMAs (partition-stride + contiguous run) stay on the
    # hardware DGE; a single 3D strided DMA would fall back to SWDGE.
    def load_pair(p):
        t = sbA.tile([128, 4, D], F32, tag="x_tm")
        for c in range(4):
            nc.sync.dma_start(out=t[:, c, :],
                              in_=x_ap[512 * p + 128 * c:512 * p + 128 * (c + 1)])
        return t

    def stage_a(x_tm):
        """x transpose to feat-major + QKV + v."""
        st = {}
        xf_ps = ps.tile([D, 512], F32, tag="ps")
        for c in range(4):
            nc.tensor.transpose(xf_ps[:, 128 * c:128 * c + 128],
                                x_tm[:, c, :], ident[:])
        x_fm = sbA.tile([D, 512], BF16, tag="x_fm")
        nc.vector.tensor_copy(x_fm[:], xf_ps[:])
        st["x_fm"] = x_fm
        q_sb, k_sb = [], []
        for r in range(2):
            q_ps = ps.tile([128, 512], F32, tag="ps")
            nc.tensor.matmul(q_ps[:], wq_sb[:, r, :], x_fm[:],
                             start=True, stop=True)
            qs = sbA.tile([128, 512], BF16, tag=f"q_sb{r}")
            nc.scalar.activation(qs[:], q_ps[:], AF.Copy)
            q_sb.append(qs)
            k_ps = ps.tile([128, 512], F32, tag="ps")
            nc.tensor.matmul(k_ps[:], wk_sb[:, r, :], x_fm[:],
                             start=True, stop=True)
            ks = sbA.tile([128, 512], BF16, tag=f"k_sb{r}")
            nc.scalar.activation(ks[:], k_ps[:], AF.Copy)
            k_sb.append(ks)
        st["q_sb"], st["k_sb"] = q_sb, k_sb
        v_ps = ps.tile([128, 4, D], F32, tag="ps")
        for c in range(4):
            nc.tensor.matmul(v_ps[:, c, :],
                             x_fm[:, 128 * c:128 * c + 128], wv_sb[:],
                             start=True, stop=True)
        v_aug = v_aug_bufs[stage_a.parity]
        stage_a.parity ^= 1
        # strided copy: v_ps [128,4,(h e)] -> v_aug cols 9h+j, ones preserved
        dst = v_aug[:, :, 0:72].rearrange("p c (h x) -> p c h x", x=9)[:, :, :, 0:8]
        nc.vector.tensor_copy(dst, v_ps[:].rearrange("p c (h x) -> p c h x", x=8))
        st["v_aug"] = v_aug
        return st
    stage_a.parity = 0

    def b1_unit(st, j, r):
        """one attention unit: scores+exp+mask+o for 4 heads of sequence j
        (proj round r). Both rounds' o (and their softmax sums on rows
        32g+8) accumulate into one [128, 2, 256] PSUM tile."""
        q_sb, k_sb, v_aug = st["q_sb"], st["k_sb"], st["v_aug"]
        tcol = slice(256 * j, 256 * j + 256)
        t0 = slice(256 * j, 256 * j + 128)
        t1 = slice(256 * j + 128, 256 * j + 256)
        # scores: 2 sub-rounds of 2 heads; each head owns one PSUM bank
        # within its [128, 2, 512] tile (bank-conflict rule)
        e_tiles = []
        for a in range(2):
            sc = psc.tile([128, 2, 512], F32, tag="sc")
            for b in range(2):
                g = 2 * a + b
                rg = slice(32 * g, 32 * g + 8)
                nc.tensor.matmul(sc[:, b, 0:256], k_sb[r][rg, t0],
                                 q_sb[r][rg, tcol],
                                 start=True, stop=True,
                                 tile_position=(32 * g, 0))
                nc.tensor.matmul(sc[:, b, 256:384], k_sb[r][rg, t1],
                                 q_sb[r][rg, t1],
                                 start=True, stop=True,
                                 tile_position=(32 * g, 0))
            e = sbB.tile([128, 2, 384], BF16, tag="e")
            nc.scalar.activation(e[:], sc[:, :, 0:384], AF.Exp)
            # causal: keep t - s >= 0 on diagonal blocks
            nc.gpsimd.affine_select(out=e[:, :, 0:128],
                                    in_=e[:, :, 0:128],
                                    compare_op=OP.is_ge, fill=0.0,
                                    base=0, pattern=[[0, 2], [1, 128]],
                                    channel_multiplier=-1)
            nc.gpsimd.affine_select(out=e[:, :, 256:384],
                                    in_=e[:, :, 256:384],
                                    compare_op=OP.is_ge, fill=0.0,
                                    base=0, pattern=[[0, 2], [1, 128]],
                                    channel_multiplier=-1)
            e_tiles.append(e)
        o_ps = ps.tile([128, 256], F32, tag="ps")
        for g in range(4):
            a, b = divmod(g, 2)
            e0 = e_tiles[a][:, b, 0:256]
            e1 = e_tiles[a][:, b, 256:384]
            hh = 4 * r + g
            cg = slice(32 * g, 32 * g + 32)
            vA = v_aug[:, 2 * j, 9 * hh:9 * hh + 32]
            vB = v_aug[:, 2 * j + 1, 9 * hh:9 * hh + 32]
            nc.tensor.matmul(o_ps[cg, :], vA, e0,
                             start=True, stop=False,
                             tile_position=(0, 32 * g))
            nc.tensor.matmul(o_ps[cg, 128:256], vB, e1,
                             start=False, stop=True,
                             tile_position=(0, 32 * g))
        # softmax denominators sit on row 32g+8 of each group; broadcast
        # them to all lanes of the group, then 1/x, then scale o (junk
        # rows are killed by zero rows in Wp).
        sums_b = sbB.tile([128, 256], F32, tag="sums_b")
        nc.vector.stream_shuffle(sums_b[:], o_ps[:], [8] * 32)
        recip = sbB.tile([128, 256], F32, tag="recip")
        nc.vector.reciprocal_approx_fast(out=recip[:], in_=sums_b[:])
        on = sbB.tile([128, 256], BF16, tag="o_sb")
        nc.vector.tensor_mul(on[:], o_ps[:], recip[:])
        st.setdefault("o_sb", {})[(j, r)] = on

    def b1_proj(st, j):
        """projection for sequence j once both its rounds' o are ready."""
        o_sb = st["o_sb"]
        pj = ps.tile([D, 256], F32, tag="ps")
        for r in range(2):
            nc.tensor.matmul(pj[:], wp_sb[:, r, :], o_sb.pop((j, r))[:],
                             start=(r == 0), stop=(r == 1))
        st.setdefault("pj_ps", {})[j] = pj

    def b1_fin(st):
        """residual 1 (feat-major) + LN1 input transposes."""
        x_fm, pj_ps = st["x_fm"], st.pop("pj_ps")
        h_pre = sbA.tile([D, 512], F32, tag="h_pre")
        for j in range(2):
            tcol = slice(256 * j, 256 * j + 256)
            nc.vector.tensor_add(h_pre[:, tcol], pj_ps[j][:], x_fm[:, tcol])
        ht_ps = ps.tile([128, 4, D], F32, tag="tm", bufs=1)
        for c in range(4):
            nc.tensor.transpose(ht_ps[:, c, :],
                                h_pre[:, 128 * c:128 * c + 128],
                                ident[0:D, 0:D])
        st["ht_ps"] = ht_ps

    def stage_b1(st):
        for j in range(2):
            for r in range(2):
                b1_unit(st, j, r)
            b1_proj(st, j)
        b1_fin(st)

    def stage_b2a(st):
        """LN1 stats + rstd scale/bias prep: scale = rstd, bias = -mu*rstd."""
        ht_ps = st["ht_ps"]
        stt = sbB.tile([128, 4, 6], F32, tag="st")
        mv = sbB.tile([128, 4, 2], F32, tag="mv")
        for c in range(4):
            nc.vector.bn_stats(stt[:, c, :], ht_ps[:, c, :])
            nc.vector.bn_aggr(mv[:, c, :], stt[:, c, :])
        # rstd = exp(-0.5*ln(var+eps)): Ln/Exp share one ACT table set with
        # Relu/Copy, avoiding ~2.7us table reloads that Sqrt would force
        sd = sbB.tile([128, 4], F32, tag="sd")
        nc.scalar.activation(sd[:], mv[:, :, 1], AF.Ln, bias=eps_t[:])
        rs = sbB.tile([128, 4], F32, tag="rs8")
        nc.scalar.activation(rs[:], sd[:], AF.Exp, scale=-0.5)
        bm = sbB.tile([128, 4], F32, tag="bm")
        nc.gpsimd.tensor_mul(bm[:], mv[:, :, 0:1], rs[:])
        b4 = sbB.tile([128, 4], F32, tag="b4")
        nc.gpsimd.tensor_scalar(out=b4[:], in0=bm[:], scalar1=-1.0,
                                scalar2=None, op0=OP.mult)
        st["rs8"], st["b4"] = rs, b4

    def stage_b2b(st):
        """LN1 apply (ScalarE Copy with per-partition scale/bias) + hhat
        back to feat-major (bf16)."""
        ht_ps, rs8, b4 = st.pop("ht_ps"), st.pop("rs8"), st.pop("b4")
        hh_tm = sbB.tile([128, 4, D], BF16, tag="hh_tm")
        for c in range(4):
            nc.scalar.activation(hh_tm[:, c, :], ht_ps[:, c, :], AF.Identity,
                                 bias=b4[:, c:c + 1], scale=rs8[:, c:c + 1])
        hf_ps = ps.tile([D, 512], BF16, tag="ps")
        for c in range(4):
            nc.tensor.transpose(hf_ps[:, 128 * c:128 * c + 128],
                                hh_tm[:, c, :], identb[:])
        # duplicate hhat into both partition halves (rows 64:128 via
        # SBUF->SBUF DMA) so FFN1 can run 2 row-tiled matmuls concurrently
        hh_bf = sbA.tile([128, 512], BF16, tag="hh_bf")
        nc.vector.tensor_copy(hh_bf[0:D, :], hf_ps[:])
        nc.sync.dma_start(out=hh_bf[D:128, :], in_=hh_bf[0:D, :])
        st["hh_bf"] = hh_bf

    def stage_d1(st, st_next):
        """FFN matmuls (FFN1 row-packed 2x) with next-pair attention units
        woven between chunk groups; relu evictions split ScalarE/DVE."""
        hh_bf = st["hh_bf"]
        hid = sbH.tile([128, NCHUNK, 512], BF16, tag="hid")
        for c in range(NCHUNK // 2):
            # chunks c and c+8 run concurrently on row groups {0,1}/{2,3},
            # each into its own PSUM bank
            fa = ps.tile([128, 512], F32, tag="ps")
            fb = ps.tile([128, 512], F32, tag="ps")
            nc.tensor.matmul(fa[:], w1_sb[0:D, c, :], hh_bf[0:D, :],
                             start=True, stop=True, tile_position=(0, 0))
            nc.tensor.matmul(fb[:], w1_sb[D:128, c, :], hh_bf[D:128, :],
                             start=True, stop=True, tile_position=(D, 0))
            for k, f in ((c, fa), (c + 8, fb)):
                if _RELU_ENG[k] == 0:
                    nc.scalar.activation(hid[:, k, :], f[:], AF.Relu)
                else:
                    nc.vector.tensor_scalar(out=hid[:, k, :], in0=f[:],
                                            scalar1=0.0, scalar2=None,
                                            op0=OP.max)
            if st_next is not None and c % 2 == 1:
                u = c // 2  # 0..3
                j, r = divmod(u, 2)
                b1_unit(st_next, j, r)
                if r == 1:
                    b1_proj(st_next, j)
        if st_next is not None:
            b1_fin(st_next)
        z_ps = ps.tile([D, 512], F32, tag="ps")
        for c in range(NCHUNK):
            nc.tensor.matmul(z_ps[:], w2_sb[:, c, :], hid[:, c, :],
                             start=(c == 0), stop=False)
        nc.tensor.matmul(z_ps[:], g1d_sb[:], hh_bf[0:D, :],
                         start=False, stop=True)
        st["z_ps"] = z_ps

    def stage_d2(st, p):
        """z evict + LN2 (reduce-based stats, DVE applies) + store."""
        z_ps = st.pop("z_ps")
        z_sb = sbA.tile([D, 512], F32, tag="z_sb")
        nc.scalar.activation(z_sb[:], z_ps[:], AF.Copy)
        zt_ps = ps.tile([128, 4, D], F32, tag="tm", bufs=1)
        for c in range(4):
            nc.tensor.transpose(zt_ps[:, c, :],
                                z_sb[:, 128 * c:128 * c + 128],
                                ident[0:D, 0:D])
        st2 = sbB.tile([128, 4, 6], F32, tag="st2")
        mv2 = sbB.tile([128, 4, 2], F32, tag="mv2")
        for c in range(4):
            nc.vector.bn_stats(st2[:, c, :], zt_ps[:, c, :])
            nc.vector.bn_aggr(mv2[:, c, :], st2[:, c, :])
        sd2 = sbB.tile([128, 4], F32, tag="sd2")
        nc.scalar.activation(sd2[:], mv2[:, :, 1], AF.Ln, bias=eps_t[:])
        rs2 = sbB.tile([128, 4], F32, tag="rs2")
        nc.scalar.activation(rs2[:], sd2[:], AF.Exp, scale=-0.5)
        bm = sbB.tile([128, 4], F32, tag="bm2")
        nc.gpsimd.tensor_mul(bm[:], mv2[:, :, 0:1], rs2[:])
        b4 = sbB.tile([128, 4], F32, tag="b42")
        nc.gpsimd.tensor_scalar(out=b4[:], in0=bm[:], scalar1=-1.0,
                                scalar2=None, op0=OP.mult)
        out_sb = sbA.tile([128, 4, D], F32, tag="out_sb")
        for c in range(4):
            nc.scalar.activation(out_sb[:, c, :], zt_ps[:, c, :], AF.Identity,
                                 bias=b4[:, c:c + 1], scale=rs2[:, c:c + 1])
        for c in range(4):
            nc.sync.dma_start(
                out=out_ap[512 * p + 128 * c:512 * p + 128 * (c + 1)],
                in_=out_sb[:, c, :])

    # Software-pipelined emission. Per-engine queues execute in emission
    # order, so the stage order here IS the schedule skeleton:
    #   b2a(i), a(i+1), b2b(i), d1(i) [b1(i+1) units woven in], d2(i)
    pair_seq = [pp for _ in range(REPEAT) for pp in range(NPAIR)]
    n = len(pair_seq)
    sts = {0: stage_a(load_pair(pair_seq[0]))}
    stage_b1(sts[0])
    for i, p in enumerate(pair_seq):
        stage_b2a(sts[i])
        if i + 1 < n:
            sts[i + 1] = stage_a(load_pair(pair_seq[i + 1]))
        stage_b2b(sts[i])
        stage_d1(sts[i], sts.get(i + 1))
        stage_d2(sts.pop(i), p)


def _prep_weights(inputs):
    f32 = lambda a: np.ascontiguousarray(np.asarray(a, np.float32))
    bf = lambda a: np.ascontiguousarray(np.asarray(a).astype(ml_dtypes.bfloat16))
    Wq, Wk, Wv, Wp = (f32(inputs[k]) for k in ("Wq", "Wk", "Wv", "Wp"))
    g1, beta1, W1, b1 = (f32(inputs[k]) for k in ("g1", "beta1", "W1", "b1"))
    W2, b2 = f32(inputs["W2"]), f32(inputs["b2"])
    g2, beta2 = f32(inputs["g2"]), f32(inputs["beta2"])
    bq, bk, bv, bp = (f32(inputs[k]) for k in ("bq", "bk", "bv", "bp"))
    for name, b in (("bq", bq), ("bk", bk), ("bv", bv), ("bp", bp),
                    ("b1", b1), ("b2", b2), ("beta1", beta1), ("beta2", beta2)):
        assert not np.any(b), f"nonzero {name} not supported by this kernel build"
    assert np.all(g2 == 1.0), "non-unit g2 not supported by this kernel build"

    sc = 1.0 / np.sqrt(E)
    wq_s = np.zeros((2, D, 128), np.float32)
    wk_s = np.zeros((2, D, 128), np.float32)
    wp_s = np.zeros((2, 128, D), np.float32)
    for r in range(2):
        for g in range(4):
            h = 4 * r + g
            wq_s[r, :, 32 * g:32 * g + 8] = Wq[h] * sc
            wk_s[r, :, 32 * g:32 * g + 8] = Wk[h]
            wp_s[r, 32 * g:32 * g + 8, :] = Wp[8 * h:8 * h + 8, :]
    wv = Wv.transpose(1, 0, 2).reshape(D, D)  # [d, (h,e)]
    w1f_flat = g1[:, None] * W1  # [64, 2048]
    # partition-half split for row-tiled FFN1: [128, 8, 128]
    w1f = np.zeros((128, NCHUNK // 2, 128), np.float32)
    for c in range(NCHUNK // 2):
        w1f[0:D, c, :] = w1f_flat[:, 128 * c:128 * (c + 1)]
        w1f[D:128, c, :] = w1f_flat[:, 128 * (c + 8):128 * (c + 9)]
    w2r = W2.reshape(NCHUNK, 128, D)
    g1d = np.diag(g1).astype(np.float32)
    ident = np.eye(128, dtype=np.float32)
    return {
        "wq_s": bf(wq_s), "wk_s": bf(wk_s), "wv": bf(wv),
        "wp_s": bf(wp_s), "w1f": bf(w1f), "w2r": bf(w2r),
        "g1d": bf(g1d), "ident": ident, "identb": bf(ident),
    }


def kernel(**inputs) -> np.ndarray:
    global LAST_RESULTS
    x = np.ascontiguousarray(np.asarray(inputs["x"], np.float32))  # [512,256,64]
    weights = _prep_weights(inputs)

    nc = _NC_CACHE.get(REPEAT)
    if nc is None:
        nc = _NC_CACHE[REPEAT] = _build_bass()
    in_maps = []
    for core in range(N_CORES):
        shard = x[core * S_PER_CORE:(core + 1) * S_PER_CORE].reshape(
            S_PER_CORE * T, D)
        m = {"x": np.ascontiguousarray(shard)}
        m.update(weights)
        in_maps.append(m)

    res = run_bass_kernel_spmd(nc, in_maps, core_ids=list(range(N_CORES)))
    LAST_RESULTS = res
    out = np.concatenate(
        [res.results[c]["out"].reshape(S_PER_CORE, T, D) for c in range(N_CORES)],
        axis=0)
    return out


# revision 81
# speedup vs baseline: 1.0201x; 1.0201x over previous
"""Trainium2 Bass kernel: post-norm transformer block (8-head causal attention
d_model=64 + 64->2048->64 FFN), B=512 T=256, fp32 I/O.

Sharding: pure data-parallel over 8 NeuronCores - 64 sequences per core,
weights replicated. No collectives.

Per-core dataflow (feat-major = feature dim on SBUF partitions, tokens free):
  x [512tok-pair, 64] --PE transpose--> x_fm [64, 512] bf16
  QKV: bf16 matmuls; q/k spread 4-heads-per-128-rows (32-row groups for the
       tile_position packing constraint), v token-major with ONES columns
       interleaved per head (9 cols/head) so each o-matmul also produces the
       softmax row-sums on PSUM row 32g+8 (no separate sums matmuls).
  scoresT[s,t] per head: bf16 matmuls K=8 row-packed via tile_position; exp
       on ScalarE; causal mask via gpsimd affine_select (Pool is the ONLY
       engine that cannot touch PSUM, so it owns the SBUF-side masks);
       softmax denominators for both proj rounds of a sequence live in one
       [128, 2, 256] PSUM tile: one DVE stream_shuffle (lane 8 of each
       32-bank -> all lanes), one reciprocal, one multiply per sequence.
  proj: bf16 matmuls + residual add with x_fm
  LN1/LN2 token-major (PE transpose, bn_stats/bn_aggr on DVE); rstd via
       Ln/Exp on ScalarE; applies on ScalarE as Identity(scale=rstd,
       bias=-mu*rstd) with the scale/bias products prepped on Pool.
  FFN: 16 bf16 W1 chunks row-packed 2x via tile_position rows 0/64 (hhat
       duplicated into both partition halves by an SBUF-SBUF DMA), ReLU
       evictions split ScalarE/DVE, FFN2 bf16 + diag(g1) residual matmul.
"""
import numpy as np
import ml_dtypes

import concourse.bass as bass
import concourse.bacc as bacc
import concourse.tile as tile
from concourse import mybir
from concourse.bass_utils import run_bass_kernel_spmd

dt = mybir.dt
F32 = dt.float32
BF16 = dt.bfloat16
AF = mybir.ActivationFunctionType
OP = mybir.AluOpType
AX = mybir.AxisListType

N_CORES = 8
B, T, D = 512, 256, 64
H, E = 8, 8
HID = 2048
NCHUNK = HID // 128  # 16
S_PER_CORE = B // N_CORES  # 64 sequences/core
NPAIR = S_PER_CORE // 2    # 32 pair iterations
EPS = 1e-5

LAST_RESULTS = None  # test.py reads exec_time_ns from here
REPEAT = 1  # test-only: run the whole body N times in one NEFF for timing
_NC_CACHE = {}

# FFN1 relu eviction engine per chunk pair-slot: 0=ScalarE 1=DVE (7/9)
_RELU_ENG = [1, 0, 1, 0, 1, 1, 0, 1, 1, 0, 1, 1, 0, 1, 0, 0]


def _build_bass():
    # All activation funcs used here (Exp, Ln, Relu, Copy, Identity) live in
    # the one table set "natural_log_exp_and_others". The default assigner
    # binds funcs to different sets and thrashes ~2.7us ACT_TABLE_LOADs;
    # restricting the table list pins a single always-resident set.
    import concourse.bacc as _bacc_mod
    _orig_gat = _bacc_mod.get_activation_tables

    def _one_set(arch):
        tabs = _orig_gat(arch)
        return {name: (fns if name == "natural_log_exp_and_others" else set())
                for name, fns in tabs.items()}

    _bacc_mod.get_activation_tables = _one_set
    try:
        return _build_bass_inner()
    finally:
        _bacc_mod.get_activation_tables = _orig_gat


def _build_bass_inner():
    nc = bacc.Bacc("TRN2", target_bir_lowering=False, debug=False)

    x_d = nc.dram_tensor("x", [S_PER_CORE * T, D], F32, kind="ExternalInput")
    wq_d = nc.dram_tensor("wq_s", [2, D, 128], BF16, kind="ExternalInput")
    wk_d = nc.dram_tensor("wk_s", [2, D, 128], BF16, kind="ExternalInput")
    wv_d = nc.dram_tensor("wv", [D, D], BF16, kind="ExternalInput")
    wp_d = nc.dram_tensor("wp_s", [2, 128, D], BF16, kind="ExternalInput")
    w1_d = nc.dram_tensor("w1f", [128, NCHUNK // 2, 128], BF16,
                          kind="ExternalInput")
    w2_d = nc.dram_tensor("w2r", [NCHUNK, 128, D], BF16, kind="ExternalInput")
    g1d_d = nc.dram_tensor("g1d", [D, D], BF16, kind="ExternalInput")
    id_d = nc.dram_tensor("ident", [128, 128], F32, kind="ExternalInput")
    idb_d = nc.dram_tensor("identb", [128, 128], BF16, kind="ExternalInput")
    out_d = nc.dram_tensor("out", [S_PER_CORE * T, D], F32, kind="ExternalOutput")

    with tile.TileContext(nc) as tc:
        import contextlib
        with contextlib.ExitStack() as ctx:
            _build_body(ctx, tc, nc, x_d, wq_d, wk_d, wv_d, wp_d, w1_d, w2_d,
                        g1d_d, id_d, idb_d, out_d)
    nc.compile()
    return nc


def _build_body(ctx, tc, nc, x_d, wq_d, wk_d, wv_d, wp_d, w1_d, w2_d,
                g1d_d, id_d, idb_d, out_d):
    const = ctx.enter_context(tc.tile_pool(name="const", bufs=1))
    # PSUM: 8 banks. ps = 1-bank hot ring (3 slots); tm = 1-bank (1 slot,
    # long-lived LN-stat tiles ht/zt); psc = 2-bank score tiles (2 slots).
    ps = ctx.enter_context(tc.tile_pool(name="ps", bufs=3, space="PSUM"))
    psc = ctx.enter_context(tc.tile_pool(name="psc", bufs=2, space="PSUM"))
    sbA = ctx.enter_context(tc.tile_pool(name="sbA", bufs=4))
    sbB = ctx.enter_context(tc.tile_pool(name="sbB", bufs=8))
    sbH = ctx.enter_context(tc.tile_pool(name="sbH", bufs=2))

    # ---- constants / weights (persistent, distinct tags in bufs=1 pool) ----
    ident = const.tile([128, 128], F32, tag="ident")
    nc.sync.dma_start(out=ident[:], in_=id_d.ap())
    identb = const.tile([128, 128], BF16, tag="identb")
    nc.sync.dma_start(out=identb[:], in_=idb_d.ap())
    eps_t = const.tile([128, 1], F32, tag="eps_t")
    nc.vector.memset(eps_t[:], EPS)
    # v with ones interleave: col 9h+j = v_h,j (j<8), col 9h+8 = 1.0 so the
    # o-matmul's M=32 window also emits softmax row-sums at out row 32g+8.
    v_aug_bufs = [const.tile([128, 4, 96], BF16, tag=f"v_aug{i}",
                             name=f"v_aug{i}") for i in range(2)]
    for t in v_aug_bufs:
        nc.vector.memset(t[:, :, 72:96], 0.0)
        for h in range(H):
            nc.vector.memset(t[:, :, 9 * h + 8:9 * h + 9], 1.0)

    wq_sb = const.tile([D, 2, 128], BF16, tag="wq_sb")
    nc.sync.dma_start(out=wq_sb[:], in_=wq_d.ap().rearrange("r d m -> d r m"))
    wk_sb = const.tile([D, 2, 128], BF16, tag="wk_sb")
    nc.sync.dma_start(out=wk_sb[:], in_=wk_d.ap().rearrange("r d m -> d r m"))
    wv_sb = const.tile([D, D], BF16, tag="wv_sb")
    nc.sync.dma_start(out=wv_sb[:], in_=wv_d.ap())
    # W1 pre-split into partition halves host-side: rows 0:64 hold chunks
    # 0..7, rows 64:128 hold chunks 8..15 (g1 folded), for 2-concurrent
    # row-tiled FFN1 matmuls (tile_position rows 0/64).
    w1_sb = const.tile([128, NCHUNK // 2, 128], BF16, tag="w1_sb")
    nc.sync.dma_start(out=w1_sb[:], in_=w1_d.ap())
    g1d_sb = const.tile([D, D], BF16, tag="g1d_sb")
    nc.sync.dma_start(out=g1d_sb[:], in_=g1d_d.ap())
    wp_sb = const.tile([128, 2, D], BF16, tag="wp_sb")
    nc.sync.dma_start(out=wp_sb[:], in_=wp_d.ap().rearrange("r p m -> p r m"))
    w2_sb = const.tile([128, NCHUNK, D], BF16, tag="w2_sb")
    nc.sync.dma_start(out=w2_sb[:], in_=w2_d.ap().rearrange("c p m -> p c m"))

    x_ap = x_d.ap()
    out_ap = out_d.ap()

    # per-chunk 2D DMAs (partition-stride + contiguous run) stay on the
    # hardware DGE; a single 3D strided DMA would fall back to SWDGE.
    def load_pair(p):
        t = sbA.tile([128, 4, D], F32, tag="x_tm")
        for c in range(4):
            nc.sync.dma_start(out=t[:, c, :],
                              in_=x_ap[512 * p + 128 * c:512 * p + 128 * (c + 1)])
        return t

    def stage_a(x_tm):
        """x transpose to feat-major + QKV + v."""
        st = {}
        xf_ps = ps.tile([D, 512], F32, tag="ps")
        for c in range(4):
            nc.tensor.transpose(xf_ps[:, 128 * c:128 * c + 128],
                                x_tm[:, c, :], ident[:])
        x_fm = sbA.tile([D, 512], BF16, tag="x_fm")
        nc.vector.tensor_copy(x_fm[:], xf_ps[:])
        st["x_fm"] = x_fm
        q_sb, k_sb = [], []
        for r in range(2):
            q_ps = ps.tile([128, 512], F32, tag="ps")
            nc.tensor.matmul(q_ps[:], wq_sb[:, r, :], x_fm[:],
                             start=True, stop=True)
            qs = sbA.tile([128, 512], BF16, tag=f"q_sb{r}")
            nc.scalar.activation(qs[:], q_ps[:], AF.Copy)
            q_sb.append(qs)
            k_ps = ps.tile([128, 512], F32, tag="ps")
            nc.tensor.matmul(k_ps[:], wk_sb[:, r, :], x_fm[:],
                             start=True, stop=True)
            ks = sbA.tile([128, 512], BF16, tag=f"k_sb{r}")
            nc.scalar.activation(ks[:], k_ps[:], AF.Copy)
            k_sb.append(ks)
        st["q_sb"], st["k_sb"] = q_sb, k_sb
        v_ps = ps.tile([128, 4, D], F32, tag="ps")
        for c in range(4):
            nc.tensor.matmul(v_ps[:, c, :],
                             x_fm[:, 128 * c:128 * c + 128], wv_sb[:],
                             start=True, stop=True)
        v_aug = v_aug_bufs[stage_a.parity]
        stage_a.parity ^= 1
        # strided copy: v_ps [128,4,(h e)] -> v_aug cols 9h+j, ones preserved
        dst = v_aug[:, :, 0:72].rearrange("p c (h x) -> p c h x", x=9)[:, :, :, 0:8]
        nc.vector.tensor_copy(dst, v_ps[:].rearrange("p c (h x) -> p c h x", x=8))
        st["v_aug"] = v_aug
        return st
    stage_a.parity = 0

    def b1_unit(st, j, r):
        """one attention unit: scores+exp+mask+o for 4 heads of sequence j
        (proj round r). Both rounds' o (and their softmax sums on rows
        32g+8) accumulate into one [128, 2, 256] PSUM tile."""
        q_sb, k_sb, v_aug = st["q_sb"], st["k_sb"], st["v_aug"]
        tcol = slice(256 * j, 256 * j + 256)
        t0 = slice(256 * j, 256 * j + 128)
        t1 = slice(256 * j + 128, 256 * j + 256)
        # scores: 2 sub-rounds of 2 heads; each head owns one PSUM bank
        # within its [128, 2, 512] tile (bank-conflict rule)
        e_tiles = []
        for a in range(2):
            sc = psc.tile([128, 2, 512], F32, tag="sc")
            for b in range(2):
                g = 2 * a + b
                rg = slice(32 * g, 32 * g + 8)
                nc.tensor.matmul(sc[:, b, 0:256], k_sb[r][rg, t0],
                                 q_sb[r][rg, tcol],
                                 start=True, stop=True,
                                 tile_position=(32 * g, 0))
                nc.tensor.matmul(sc[:, b, 256:384], k_sb[r][rg, t1],
                                 q_sb[r][rg, t1],
                                 start=True, stop=True,
                                 tile_position=(32 * g, 0))
            e = sbB.tile([128, 2, 384], BF16, tag="e")
            nc.scalar.activation(e[:], sc[:, :, 0:384], AF.Exp)
            # causal: keep t - s >= 0 on diagonal blocks
            nc.gpsimd.affine_select(out=e[:, :, 0:128],
                                    in_=e[:, :, 0:128],
                                    compare_op=OP.is_ge, fill=0.0,
                                    base=0, pattern=[[0, 2], [1, 128]],
                                    channel_multiplier=-1)
            nc.gpsimd.affine_select(out=e[:, :, 256:384],
                                    in_=e[:, :, 256:384],
                                    compare_op=OP.is_ge, fill=0.0,
                                    base=0, pattern=[[0, 2], [1, 128]],
                                    channel_multiplier=-1)
            e_tiles.append(e)
        if r == 0:
            st["o_ps"] = ps.tile([128, 2, 256], F32, tag="tm", bufs=1,
                                 name="o_ps")
        o_ps = st["o_ps"]
        for g in range(4):
            a, b = divmod(g, 2)
            e0 = e_tiles[a][:, b, 0:256]
            e1 = e_tiles[a][:, b, 256:384]
            hh = 4 * r + g
            cg = slice(32 * g, 32 * g + 32)
            vA = v_aug[:, 2 * j, 9 * hh:9 * hh + 32]
            vB = v_aug[:, 2 * j + 1, 9 * hh:9 * hh + 32]
            nc.tensor.matmul(o_ps[cg, r, :], vA, e0,
                             start=True, stop=False,
                             tile_position=(0, 32 * g))
            nc.tensor.matmul(o_ps[cg, r, 128:256], vB, e1,
                             start=False, stop=True,
                             tile_position=(0, 32 * g))

    def b1_proj(st, j):
        """softmax normalization for both rounds of sequence j + proj.
        Denominators sit on row 32g+8 of each group; broadcast to all
        lanes of the group, 1/x, scale o (junk rows die on zero Wp rows)."""
        o_ps = st.pop("o_ps")
        sums_b = sbB.tile([128, 2, 256], F32, tag="sums_b")
        nc.vector.stream_shuffle(sums_b[:], o_ps[:], [8] * 32)
        recip = sbB.tile([128, 2, 256], F32, tag="recip")
        nc.vector.reciprocal_approx_fast(out=recip[:], in_=sums_b[:])
        on = sbB.tile([128, 2, 256], BF16, tag="o_sb")
        nc.vector.tensor_mul(on[:], o_ps[:], recip[:])
        pj = ps.tile([D, 256], F32, tag="ps")
        for r in range(2):
            nc.tensor.matmul(pj[:], wp_sb[:, r, :], on[:, r, :],
                             start=(r == 0), stop=(r == 1))
        st.setdefault("pj_ps", {})[j] = pj

    def b1_fin(st):
        """residual 1 (feat-major) + LN1 input transposes."""
        x_fm, pj_ps = st["x_fm"], st.pop("pj_ps")
        h_pre = sbA.tile([D, 512], F32, tag="h_pre")
        for j in range(2):
            tcol = slice(256 * j, 256 * j + 256)
            nc.vector.tensor_add(h_pre[:, tcol], pj_ps[j][:], x_fm[:, tcol])
        ht_ps = ps.tile([128, 4, D], F32, tag="tm", bufs=1)
        for c in range(4):
            nc.tensor.transpose(ht_ps[:, c, :],
                                h_pre[:, 128 * c:128 * c + 128],
                                ident[0:D, 0:D])
        st["ht_ps"] = ht_ps

    def stage_b1(st):
        for j in range(2):
            for r in range(2):
                b1_unit(st, j, r)
            b1_proj(st, j)
        b1_fin(st)

    def stage_b2a(st):
        """LN1 stats + rstd scale/bias prep: scale = rstd, bias = -mu*rstd."""
        ht_ps = st["ht_ps"]
        stt = sbB.tile([128, 4, 6], F32, tag="st")
        mv = sbB.tile([128, 4, 2], F32, tag="mv")
        for c in range(4):
            nc.vector.bn_stats(stt[:, c, :], ht_ps[:, c, :])
            nc.vector.bn_aggr(mv[:, c, :], stt[:, c, :])
        # rstd = exp(-0.5*ln(var+eps)): Ln/Exp share one ACT table set with
        # Relu/Copy, avoiding ~2.7us table reloads that Sqrt would force
        sd = sbB.tile([128, 4], F32, tag="sd")
        nc.scalar.activation(sd[:], mv[:, :, 1], AF.Ln, bias=eps_t[:])
        rs = sbB.tile([128, 4], F32, tag="rs8")
        nc.scalar.activation(rs[:], sd[:], AF.Exp, scale=-0.5)
        bm = sbB.tile([128, 4], F32, tag="bm")
        nc.gpsimd.tensor_mul(bm[:], mv[:, :, 0:1], rs[:])
        b4 = sbB.tile([128, 4], F32, tag="b4")
        nc.gpsimd.tensor_scalar(out=b4[:], in0=bm[:], scalar1=-1.0,
                                scalar2=None, op0=OP.mult)
        st["rs8"], st["b4"] = rs, b4

    def stage_b2b(st):
        """LN1 apply (ScalarE Copy with per-partition scale/bias) + hhat
        back to feat-major (bf16)."""
        ht_ps, rs8, b4 = st.pop("ht_ps"), st.pop("rs8"), st.pop("b4")
        hh_tm = sbB.tile([128, 4, D], BF16, tag="hh_tm")
        for c in range(4):
            nc.scalar.activation(hh_tm[:, c, :], ht_ps[:, c, :], AF.Identity,
                                 bias=b4[:, c:c + 1], scale=rs8[:, c:c + 1])
        hf_ps = ps.tile([D, 512], BF16, tag="ps")
        for c in range(4):
            nc.tensor.transpose(hf_ps[:, 128 * c:128 * c + 128],
                                hh_tm[:, c, :], identb[:])
        # duplicate hhat into both partition halves (rows 64:128 via
        # SBUF->SBUF DMA) so FFN1 can run 2 row-tiled matmuls concurrently
        hh_bf = sbA.tile([128, 512], BF16, tag="hh_bf")
        nc.vector.tensor_copy(hh_bf[0:D, :], hf_ps[:])
        nc.sync.dma_start(out=hh_bf[D:128, :], in_=hh_bf[0:D, :])
        st["hh_bf"] = hh_bf

    def stage_d1(st, st_next):
        """FFN matmuls (FFN1 row-packed 2x) with next-pair attention units
        woven between chunk groups; relu evictions split ScalarE/DVE."""
        hh_bf = st["hh_bf"]
        hid = sbH.tile([128, NCHUNK, 512], BF16, tag="hid")
        for c in range(NCHUNK // 2):
            # chunks c and c+8 run concurrently on row groups {0,1}/{2,3},
            # each into its own PSUM bank
            fa = ps.tile([128, 512], F32, tag="ps")
            fb = ps.tile([128, 512], F32, tag="ps")
            nc.tensor.matmul(fa[:], w1_sb[0:D, c, :], hh_bf[0:D, :],
                             start=True, stop=True, tile_position=(0, 0))
            nc.tensor.matmul(fb[:], w1_sb[D:128, c, :], hh_bf[D:128, :],
                             start=True, stop=True, tile_position=(D, 0))
            for k, f in ((c, fa), (c + 8, fb)):
                if _RELU_ENG[k] == 0:
                    nc.scalar.activation(hid[:, k, :], f[:], AF.Relu)
                else:
                    nc.vector.tensor_scalar(out=hid[:, k, :], in0=f[:],
                                            scalar1=0.0, scalar2=None,
                                            op0=OP.max)
            if st_next is not None and c % 2 == 1:
                u = c // 2  # 0..3
                j, r = divmod(u, 2)
                b1_unit(st_next, j, r)
                if r == 1:
                    b1_proj(st_next, j)
        if st_next is not None:
            b1_fin(st_next)
        z_ps = ps.tile([D, 512], F32, tag="ps")
        for c in range(NCHUNK):
            nc.tensor.matmul(z_ps[:], w2_sb[:, c, :], hid[:, c, :],
                             start=(c == 0), stop=False)
        nc.tensor.matmul(z_ps[:], g1d_sb[:], hh_bf[0:D, :],
                         start=False, stop=True)
        st["z_ps"] = z_ps

    def stage_d2(st, p):
        """z evict + LN2 (reduce-based stats, DVE applies) + store."""
        z_ps = st.pop("z_ps")
        z_sb = sbA.tile([D, 512], F32, tag="z_sb")
        nc.scalar.activation(z_sb[:], z_ps[:], AF.Copy)
        zt_ps = ps.tile([128, 4, D], F32, tag="tm", bufs=1)
        for c in range(4):
            nc.tensor.transpose(zt_ps[:, c, :],
                                z_sb[:, 128 * c:128 * c + 128],
                                ident[0:D, 0:D])
        st2 = sbB.tile([128, 4, 6], F32, tag="st2")
        mv2 = sbB.tile([128, 4, 2], F32, tag="mv2")
        for c in range(4):
            nc.vector.bn_stats(st2[:, c, :], zt_ps[:, c, :])
            nc.vector.bn_aggr(mv2[:, c, :], st2[:, c, :])
        sd2 = sbB.tile([128, 4], F32, tag="sd2")
        nc.scalar.activation(sd2[:], mv2[:, :, 1], AF.Ln, bias=eps_t[:])
        rs2 = sbB.tile([128, 4], F32, tag="rs2")
        nc.scalar.activation(rs2[:], sd2[:], AF.Exp, scale=-0.5)
        bm = sbB.tile([128, 4], F32, tag="bm2")
        nc.gpsimd.tensor_mul(bm[:], mv2[:, :, 0:1], rs2[:])
        b4 = sbB.tile([128, 4], F32, tag="b42")
        nc.gpsimd.tensor_scalar(out=b4[:], in0=bm[:], scalar1=-1.0,
                                scalar2=None, op0=OP.mult)
        out_sb = sbA.tile([128, 4, D], F32, tag="out_sb")
        for c in range(4):
            nc.vector.tensor_scalar(out=out_sb[:, c, :], in0=zt_ps[:, c, :],
                                    scalar1=rs2[:, c:c + 1],
                                    scalar2=b4[:, c:c + 1],
                                    op0=OP.mult, op1=OP.add)
        for c in range(4):
            nc.sync.dma_start(
                out=out_ap[512 * p + 128 * c:512 * p + 128 * (c + 1)],
                in_=out_sb[:, c, :])

    # Software-pipelined emission. Per-engine queues execute in emission
    # order, so the stage order here IS the schedule skeleton:
    #   b2a(i), a(i+1), b2b(i), d1(i) [b1(i+1) units woven in], d2(i)
    pair_seq = [pp for _ in range(REPEAT) for pp in range(NPAIR)]
    n = len(pair_seq)
    sts = {0: stage_a(load_pair(pair_seq[0]))}
    stage_b1(sts[0])
    for i, p in enumerate(pair_seq):
        stage_b2a(sts[i])
        if i + 1 < n:
            sts[i + 1] = stage_a(load_pair(pair_seq[i + 1]))
        stage_b2b(sts[i])
        stage_d1(sts[i], sts.get(i + 1))
        stage_d2(sts.pop(i), p)


def _prep_weights(inputs):
    f32 = lambda a: np.ascontiguousarray(np.asarray(a, np.float32))
    bf = lambda a: np.ascontiguousarray(np.asarray(a).astype(ml_dtypes.bfloat16))
    Wq, Wk, Wv, Wp = (f32(inputs[k]) for k in ("Wq", "Wk", "Wv", "Wp"))
    g1, beta1, W1, b1 = (f32(inputs[k]) for k in ("g1", "beta1", "W1", "b1"))
    W2, b2 = f32(inputs["W2"]), f32(inputs["b2"])
    g2, beta2 = f32(inputs["g2"]), f32(inputs["beta2"])
    bq, bk, bv, bp = (f32(inputs[k]) for k in ("bq", "bk", "bv", "bp"))
    for name, b in (("bq", bq), ("bk", bk), ("bv", bv), ("bp", bp),
                    ("b1", b1), ("b2", b2), ("beta1", beta1), ("beta2", beta2)):
        assert not np.any(b), f"nonzero {name} not supported by this kernel build"
    assert np.all(g2 == 1.0), "non-unit g2 not supported by this kernel build"

    sc = 1.0 / np.sqrt(E)
    wq_s = np.zeros((2, D, 128), np.float32)
    wk_s = np.zeros((2, D, 128), np.float32)
    wp_s = np.zeros((2, 128, D), np.float32)
    for r in range(2):
        for g in range(4):
            h = 4 * r + g
            wq_s[r, :, 32 * g:32 * g + 8] = Wq[h] * sc
            wk_s[r, :, 32 * g:32 * g + 8] = Wk[h]
            wp_s[r, 32 * g:32 * g + 8, :] = Wp[8 * h:8 * h + 8, :]
    wv = Wv.transpose(1, 0, 2).reshape(D, D)  # [d, (h,e)]
    w1f_flat = g1[:, None] * W1  # [64, 2048]
    # partition-half split for row-tiled FFN1: [128, 8, 128]
    w1f = np.zeros((128, NCHUNK // 2, 128), np.float32)
    for c in range(NCHUNK // 2):
        w1f[0:D, c, :] = w1f_flat[:, 128 * c:128 * (c + 1)]
        w1f[D:128, c, :] = w1f_flat[:, 128 * (c + 8):128 * (c + 9)]
    w2r = W2.reshape(NCHUNK, 128, D)
    g1d = np.diag(g1).astype(np.float32)
    ident = np.eye(128, dtype=np.float32)
    return {
        "wq_s": bf(wq_s), "wk_s": bf(wk_s), "wv": bf(wv),
        "wp_s": bf(wp_s), "w1f": bf(w1f), "w2r": bf(w2r),
        "g1d": bf(g1d), "ident": ident, "identb": bf(ident),
    }


def kernel(**inputs) -> np.ndarray:
    global LAST_RESULTS
    x = np.ascontiguousarray(np.asarray(inputs["x"], np.float32))  # [512,256,64]
    weights = _prep_weights(inputs)

    nc = _NC_CACHE.get(REPEAT)
    if nc is None:
        nc = _NC_CACHE[REPEAT] = _build_bass()
    in_maps = []
    for core in range(N_CORES):
        shard = x[core * S_PER_CORE:(core + 1) * S_PER_CORE].reshape(
            S_PER_CORE * T, D)
        m = {"x": np.ascontiguousarray(shard)}
        m.update(weights)
        in_maps.append(m)

    res = run_bass_kernel_spmd(nc, in_maps, core_ids=list(range(N_CORES)))
    LAST_RESULTS = res
    out = np.concatenate(
        [res.results[c]["out"].reshape(S_PER_CORE, T, D) for c in range(N_CORES)],
        axis=0)
    return out
